# revision 8
# baseline (speedup 1.0000x reference)
"""Multi-head causal self-attention (B=4, S=2048, E=1024, H=16, D=64) on 8 TRN2 cores.

Sharding: data-parallel over batch (4 batches x 2 cores each); within a batch
pair, queries are split causally-balanced (zigzag q-blocks) so both cores do
equal attention work with zero cross-core communication.  Each core computes
K/V for the full sequence of its batch (all heads), Q for its own 1024 rows,
causal attention, and the output projection for its own rows.  The host only
shards inputs and scatters the disjoint output rows back.

Compute dtype: float16 matmul operands, fp32 PSUM accumulation, exp on ACT in
fp32 with per-(span,block) scale/bias vectors implementing block-level causal
kills; triangular diagonal masks are additive f32 tiles from the host.
"""

import os
import numpy as np

import concourse.mybir as mybir
import concourse.tile as tile
import concourse.bacc as bacc
from concourse.bass_utils import run_bass_kernel_spmd

B, S, E, H, D = 4, 2048, 1024, 16, 64
KB = S // 128            # 16 kv blocks of 128
NSPAN = 4                # q spans of 256 per core
F32 = mybir.dt.float32
F16 = mybir.dt.float16

# q-block (128-row) assignment per half, grouped into 4 spans of 2 blocks.
OWN_BLOCKS = {
    0: [[0, 1], [6, 7], [8, 9], [14, 15]],
    1: [[2, 3], [4, 5], [10, 11], [12, 13]],
}
BOUNDS = [4, 8, 12, 16]          # uniform kv-block bound per span slot
MASK_OFFS = [0, 128, 384, 512]   # packed col offsets of the 4 masked blocks
MASK_W = [128, 256, 128, 256]    # widths (m%2+1)*128


def own_rows(half):
    return np.concatenate(
        [np.arange(b * 128, b * 128 + 128) for sp in OWN_BLOCKS[half] for b in sp]
    )


def _build_nc():
    nc = bacc.Bacc("TRN2", target_bir_lowering=False, debug=False, num_devices=8)
    xt = nc.dram_tensor("xt", [E, S], F32, kind="ExternalInput")
    xqt = nc.dram_tensor("xqt", [E, S // 2], F32, kind="ExternalInput")
    wqt = nc.dram_tensor("wqt", [E, E], F32, kind="ExternalInput")
    wkt = nc.dram_tensor("wkt", [E, E], F32, kind="ExternalInput")
    wvt = nc.dram_tensor("wvt", [E, E], F32, kind="ExternalInput")
    wp = nc.dram_tensor("wp", [E, E], F32, kind="ExternalInput")
    bpv = nc.dram_tensor("bpv", [1, E], F32, kind="ExternalInput")
    masks = nc.dram_tensor("masks", [128, NSPAN, 768], F32, kind="ExternalInput")
    scv_d = nc.dram_tensor("scv", [128, NSPAN, KB], F32, kind="ExternalInput")
    biv_d = nc.dram_tensor("biv", [128, NSPAN, KB], F32, kind="ExternalInput")
    out = nc.dram_tensor("out", [S // 2, E], F32, kind="ExternalOutput")

    xt_r = xt.rearrange("(c p) s -> p c s", p=128)
    xqt_r = xqt.rearrange("(c p) s -> p c s", p=128)
    wqt_r = wqt.rearrange("(c p) n -> p c n", p=128)
    wkt_r = wkt.rearrange("(c p) n -> p c n", p=128)
    wvt_r = wvt.rearrange("(c p) n -> p c n", p=128)
    wp_r = wp.rearrange("(c p) n -> p c n", p=128)

    with tile.TileContext(nc) as tc:
        with tc.tile_pool(name="persist", bufs=1) as pers, \
             tc.tile_pool(name="ph1", bufs=2) as ph1, \
             tc.tile_pool(name="ph2", bufs=3) as ph2, \
             tc.tile_pool(name="ph2c", bufs=4) as ph2c, \
             tc.tile_pool(name="ph3", bufs=2) as ph3, \
             tc.tile_pool(name="psA", bufs=2, space="PSUM") as psA, \
             tc.tile_pool(name="psS", bufs=3, space="PSUM") as psS, \
             tc.tile_pool(name="psC", bufs=1, space="PSUM") as psC, \
             tc.tile_pool(name="psO", bufs=1, space="PSUM") as psO, \
             tc.tile_pool(name="dram", bufs=4, space="DRAM") as dram:

            # persistent split tiles: chunk granularity so attention/proj can
            # start as soon as their inputs exist.
            KT = [[pers.tile([128, 512], F16, tag=f"kt{i}_{c}", name=f"kt{i}_{c}")
                   for c in range(4)] for i in range(8)]
            QT = [[pers.tile([128, 512], F16, tag=f"qt{i}_{c}", name=f"qt{i}_{c}")
                   for c in range(2)] for i in range(8)]
            VA = [pers.tile([128, 4, H, 65], F16, tag=f"va{c}", name=f"va{c}")
                  for c in range(4)]
            CN = [[pers.tile([128, 256], F16, tag=f"cn{i}_{c}", name=f"cn{i}_{c}")
                   for c in range(NSPAN)] for i in range(8)]

            for c in range(4):
                nc.gpsimd.memset(VA[c][:, :, :, 64:65], 1.0)

            mk = pers.tile([128, NSPAN, 768], F32)
            nc.sync.dma_start(mk[:], masks[:])
            scv = pers.tile([128, NSPAN, KB], F32)
            nc.sync.dma_start(scv[:], scv_d[:])
            biv = pers.tile([128, NSPAN, KB], F32)
            nc.sync.dma_start(biv[:], biv_d[:])
            wpt = pers.tile([128, 8, E], F16)
            bpb = pers.tile([128, E], F32)

            def emit_qt(qh):
                xqs = ph1.tile([128, 8, 512], F16, tag="xts")
                for ec in range(8):
                    nc.gpsimd.dma_start(xqs[:, ec, :], xqt_r[:, ec, qh * 512:(qh + 1) * 512])
                for hp in range(8):
                    wq_t = ph1.tile([128, 8, 128], F16, tag="wk")
                    for ec in range(8):
                        nc.gpsimd.dma_start(wq_t[:, ec, :], wqt_r[:, ec, hp * 128:(hp + 1) * 128])
                    qps = psA.tile([128, 512], F32, tag="pps")
                    for ec in range(8):
                        nc.tensor.matmul(qps[:], wq_t[:, ec, :], xqs[:, ec, :],
                                         start=(ec == 0), stop=(ec == 7))
                    nc.vector.tensor_copy(QT[hp][qh][:], qps[:])

            def emit_kv(sp4):
                xts = ph1.tile([128, 8, 512], F16, tag="xts")
                for ec in range(8):
                    nc.gpsimd.dma_start(xts[:, ec, :], xt_r[:, ec, sp4 * 512:(sp4 + 1) * 512])
                for hp in range(8):
                    wk_t = ph1.tile([128, 8, 128], F16, tag="wk")
                    for ec in range(8):
                        nc.gpsimd.dma_start(wk_t[:, ec, :], wkt_r[:, ec, hp * 128:(hp + 1) * 128])
                    kps = psA.tile([128, 512], F32, tag="pps")
                    for ec in range(8):
                        nc.tensor.matmul(kps[:], wk_t[:, ec, :], xts[:, ec, :],
                                         start=(ec == 0), stop=(ec == 7))
                    nc.vector.tensor_copy(KT[hp][sp4][:], kps[:])
                for hh in range(2):
                    wv_t = ph1.tile([128, 8, 512], F16, tag="wv")
                    for ec in range(8):
                        nc.gpsimd.dma_start(wv_t[:, ec, :], wvt_r[:, ec, hh * 512:(hh + 1) * 512])
                    for j in range(4):
                        vps = psA.tile([128, 512], F32, tag="pps")
                        for ec in range(8):
                            nc.tensor.matmul(vps[:], xts[:, ec, j * 128:(j + 1) * 128],
                                             wv_t[:, ec, :],
                                             start=(ec == 0), stop=(ec == 7))
                        for hl in range(8):
                            h = hh * 8 + hl
                            nc.vector.tensor_copy(VA[sp4][:, j, h, 0:64],
                                                  vps[:, hl * 64:hl * 64 + 64])

            def emit_span(s):
                Bs = BOUNDS[s]
                q0 = s * 256
                qh, qo = s // 2, (s % 2) * 256
                for gg in range(8):
                    # group of 2 same-parity heads: j = gg // 2, p = gg % 2
                    # heads (4j+p, 4j+2+p) -> KT/QT pair rows p*64..p*64+64
                    j, p = gg // 2, gg % 2
                    hA, hB = 4 * j + p, 4 * j + 2 + p
                    r0 = p * 64
                    tp = (64, 0) if p else None
                    cpsA = psC.tile([65, 256], F32, tag="cpsA")
                    cpsB = psC.tile([65, 256], F32, tag="cpsB")
                    for kb in range(Bs):
                        sps = psS.tile([128, 2, 256], F32, tag="sps")
                        c4, k0 = kb // 4, (kb % 4) * 128
                        nc.tensor.matmul(sps[:, 0, :],
                                         KT[2 * j][c4][r0:r0 + 64, k0:k0 + 128],
                                         QT[2 * j][qh][r0:r0 + 64, qo:qo + 256],
                                         start=True, stop=True, tile_position=tp)
                        nc.tensor.matmul(sps[:, 1, :],
                                         KT[2 * j + 1][c4][r0:r0 + 64, k0:k0 + 128],
                                         QT[2 * j + 1][qh][r0:r0 + 64, qo:qo + 256],
                                         start=True, stop=True, tile_position=tp)
                        m = kb - (Bs - 4)
                        if m >= 0:
                            w = MASK_W[m]
                            off = MASK_OFFS[m]
                            mkap = mk[:, s, off:off + w] \
                                .rearrange("p (o w) -> p o w", o=1) \
                                .to_broadcast((128, 2, w))
                            nc.vector.tensor_add(sps[:, 0:2, 0:w], sps[:, 0:2, 0:w], mkap)
                        pt = ph2.tile([128, 2, 256], F16, tag="pt")
                        nc.scalar.activation(pt[:], sps[:], mybir.ActivationFunctionType.Exp,
                                             scale=scv[:, s, kb:kb + 1],
                                             bias=biv[:, s, kb:kb + 1])
                        st, en = (kb == 0), (kb == Bs - 1)
                        nc.tensor.matmul(cpsA[:], VA[c4][:, kb % 4, hA, :], pt[:, 0, :],
                                         start=st, stop=en)
                        nc.tensor.matmul(cpsB[:], VA[c4][:, kb % 4, hB, :], pt[:, 1, :],
                                         start=st, stop=en)
                    # stage out of PSUM quickly, then normalize from SBUF
                    ctxu = ph2c.tile([65, 2, 256], F32, tag="ctxu")
                    nc.vector.tensor_copy(ctxu[:, 0, :], cpsA[:])
                    nc.vector.tensor_copy(ctxu[:, 1, :], cpsB[:])
                    rs = ph2c.tile([1, 2, 256], F32, tag="rs")
                    nc.vector.tensor_copy(rs[0:1, 0, :], ctxu[64:65, 0, :])
                    nc.vector.tensor_copy(rs[0:1, 1, :], ctxu[64:65, 1, :])
                    rr = ph2c.tile([1, 2, 256], F32, tag="rr")
                    nc.vector.reciprocal_approx_fast(rr[:], rs[:])
                    rd = dram.tile([1, 2, 256], F32, tag="rd")
                    nc.sync.dma_start(rd[:], rr[:])
                    for i, h in ((0, hA), (1, hB)):
                        bct = ph2c.tile([64, 256], F32, tag="bct")
                        nc.sync.dma_start(bct[:], rd[0:1, i, :].to_broadcast((64, 256)))
                        hp2, rr0 = h // 2, (h % 2) * 64
                        nc.vector.tensor_mul(CN[hp2][s][rr0:rr0 + 64, :],
                                             ctxu[0:64, i, :], bct[:])

            def emit_proj(s):
                for qb in (2 * s, 2 * s + 1):
                    for eo in range(2):
                        ops = psO.tile([128, 512], F32, tag="ops")
                        for c in range(8):
                            nc.tensor.matmul(ops[:],
                                             CN[c][s][:, (qb % 2) * 128:(qb % 2) * 128 + 128],
                                             wpt[:, c, eo * 512:(eo + 1) * 512],
                                             start=(c == 0), stop=(c == 7))
                        ot = ph3.tile([128, 512], F32, tag="ot")
                        nc.vector.tensor_add(ot[:], ops[:], bpb[:, eo * 512:(eo + 1) * 512])
                        nc.sync.dma_start(out[qb * 128:(qb + 1) * 128,
                                              eo * 512:(eo + 1) * 512], ot[:])

            emit_qt(0)
            emit_kv(0)
            for wc in range(8):
                nc.gpsimd.dma_start(wpt[:, wc, :], wp_r[:, wc, :])
            nc.sync.dma_start(bpb[:], bpv[0:1, :].to_broadcast((128, E)))
            emit_span(0)
            emit_proj(0)
            emit_kv(1)
            emit_span(1)
            emit_proj(1)
            emit_qt(1)
            emit_kv(2)
            emit_span(2)
            emit_proj(2)
            emit_kv(3)
            emit_span(3)
            emit_proj(3)
    nc.compile()
    return nc


_NC_CACHE = None


def _host_side_tables(half):
    """Triangular masks, scale and bias vectors for one core half."""
    mask = np.zeros((128, NSPAN, 768), np.float32)
    scv = np.zeros((128, NSPAN, KB), np.float32)
    biv = np.zeros((128, NSPAN, KB), np.float32)
    for s in range(NSPAN):
        Bs = BOUNDS[s]
        gmax = OWN_BLOCKS[half][s][1]
        for kb in range(Bs):
            if kb > gmax:
                scv[:, s, kb] = 0.0
                biv[:, s, kb] = -30.0
            else:
                scv[:, s, kb] = 1.0 / np.sqrt(D)
                biv[:, s, kb] = 0.0
        for m in range(4):
            kb = Bs - 4 + m
            w = MASK_W[m]
            off = MASK_OFFS[m]
            nqb = m % 2 + 1   # q-blocks covered by this mask
            for jj in range(nqb):
                g = OWN_BLOCKS[half][s][jj]
                kpos = kb * 128 + np.arange(128)[:, None]
                qpos = g * 128 + np.arange(128)[None, :]
                mask[:, s, off + jj * 128: off + (jj + 1) * 128] = \
                    np.where(qpos >= kpos, 0.0, -240.0)
    return mask, scv, biv


def kernel(x, Wq, Wk, Wv, Wp, bp):
    global _NC_CACHE
    x = np.asarray(x, np.float32)
    Wq = np.asarray(Wq, np.float32)
    Wk = np.asarray(Wk, np.float32)
    Wv = np.asarray(Wv, np.float32)
    Wp = np.asarray(Wp, np.float32)
    bp = np.asarray(bp, np.float32)

    if _NC_CACHE is None:
        _NC_CACHE = _build_nc()
    nc = _NC_CACHE

    wqt = np.ascontiguousarray(Wq.transpose(1, 0, 2).reshape(E, E))
    wkt = np.ascontiguousarray(Wk.transpose(1, 0, 2).reshape(E, E))
    wvt = np.ascontiguousarray(Wv.transpose(1, 0, 2).reshape(E, E))
    wp_c = np.ascontiguousarray(Wp)
    bpv = bp.reshape(1, E)
    tables = {h: _host_side_tables(h) for h in (0, 1)}
    rows = {h: own_rows(h) for h in (0, 1)}

    in_maps = []
    for c in range(8):
        b, h = c // 2, c % 2
        xb = x[b]
        mask, scv, biv = tables[h]
        in_maps.append({
            "xt": np.ascontiguousarray(xb.T),
            "xqt": np.ascontiguousarray(xb[rows[h]].T),
            "wqt": wqt, "wkt": wkt, "wvt": wvt, "wp": wp_c, "bpv": bpv,
            "masks": mask, "scv": scv, "biv": biv,
        })

    trace = bool(os.environ.get("BASS_ATTN_TRACE"))
    res = run_bass_kernel_spmd(nc, in_maps, core_ids=list(range(8)), trace=trace)
    if trace and res.exec_time_ns is not None:
        print(f"HW exec time: {res.exec_time_ns} ns")
        for scope, cores in sorted((res.per_core_scope_times or {}).items()):
            print("scope", scope, cores)
        if res.instructions_and_trace:
            print("trace path:", res.instructions_and_trace[1])

    out = np.empty((B, S, E), np.float32)
    for c in range(8):
        b, h = c // 2, c % 2
        out[b, rows[h]] = res.results[c]["out"]
    return out


# revision 9
# speedup vs baseline: 1.1058x; 1.1058x over previous
"""Multi-head causal self-attention (B=4, S=2048, E=1024, H=16, D=64) on 8 TRN2 cores.

Sharding: data-parallel over batch (4 batches x 2 cores each); within a batch
pair, queries are split causally-balanced (zigzag q-blocks) so both cores do
equal attention work with zero cross-core communication.  Each core computes
K/V for the full sequence of its batch (all heads), Q for its own 1024 rows,
causal attention, and the output projection for its own rows.  The host only
shards inputs and scatters the disjoint output rows back.

Compute dtype: float16 matmul operands, fp32 PSUM accumulation, exp on ACT in
fp32 with per-(span,block) scale/bias vectors implementing block-level causal
kills; triangular diagonal masks are additive f32 tiles from the host.
"""

import os
import numpy as np

import concourse.mybir as mybir
import concourse.tile as tile
import concourse.bacc as bacc
from concourse.bass_utils import run_bass_kernel_spmd

B, S, E, H, D = 4, 2048, 1024, 16, 64
KB = S // 128            # 16 kv blocks of 128
NSPAN = 4                # q spans of 256 per core
F32 = mybir.dt.float32
F16 = mybir.dt.float16

# q-block (128-row) assignment per half, grouped into 4 spans of 2 blocks.
OWN_BLOCKS = {
    0: [[0, 1], [6, 7], [8, 9], [14, 15]],
    1: [[2, 3], [4, 5], [10, 11], [12, 13]],
}
BOUNDS = [4, 8, 12, 16]          # uniform kv-block bound per span slot
MASK_OFFS = [0, 128, 384, 512]   # packed col offsets of the 4 masked blocks
MASK_W = [128, 256, 128, 256]    # widths (m%2+1)*128


def own_rows(half):
    return np.concatenate(
        [np.arange(b * 128, b * 128 + 128) for sp in OWN_BLOCKS[half] for b in sp]
    )


def _build_nc():
    nc = bacc.Bacc("TRN2", target_bir_lowering=False, debug=False, num_devices=8)
    xt = nc.dram_tensor("xt", [E, S], F32, kind="ExternalInput")
    xqt = nc.dram_tensor("xqt", [E, S // 2], F32, kind="ExternalInput")
    wqt = nc.dram_tensor("wqt", [E, E], F32, kind="ExternalInput")
    wkt = nc.dram_tensor("wkt", [E, E], F32, kind="ExternalInput")
    wvt = nc.dram_tensor("wvt", [E, E], F32, kind="ExternalInput")
    wp = nc.dram_tensor("wp", [E, E], F32, kind="ExternalInput")
    bpv = nc.dram_tensor("bpv", [1, E], F32, kind="ExternalInput")
    masks = nc.dram_tensor("masks", [128, NSPAN, 768], F32, kind="ExternalInput")
    scv_d = nc.dram_tensor("scv", [128, NSPAN, KB], F32, kind="ExternalInput")
    biv_d = nc.dram_tensor("biv", [128, NSPAN, KB], F32, kind="ExternalInput")
    out = nc.dram_tensor("out", [S // 2, E], F32, kind="ExternalOutput")

    xt_r = xt.rearrange("(c p) s -> p c s", p=128)
    xqt_r = xqt.rearrange("(c p) s -> p c s", p=128)
    wqt_r = wqt.rearrange("(c p) n -> p c n", p=128)
    wkt_r = wkt.rearrange("(c p) n -> p c n", p=128)
    wvt_r = wvt.rearrange("(c p) n -> p c n", p=128)
    wp_r = wp.rearrange("(c p) n -> p c n", p=128)

    with tile.TileContext(nc) as tc:
        with tc.tile_pool(name="persist", bufs=1) as pers, \
             tc.tile_pool(name="ph1", bufs=2) as ph1, \
             tc.tile_pool(name="ph2", bufs=3) as ph2, \
             tc.tile_pool(name="ph2c", bufs=4) as ph2c, \
             tc.tile_pool(name="ph3", bufs=2) as ph3, \
             tc.tile_pool(name="psA", bufs=2, space="PSUM") as psA, \
             tc.tile_pool(name="psS", bufs=3, space="PSUM") as psS, \
             tc.tile_pool(name="psC", bufs=1, space="PSUM") as psC, \
             tc.tile_pool(name="psO", bufs=1, space="PSUM") as psO, \
             tc.tile_pool(name="dram", bufs=4, space="DRAM") as dram:

            # persistent split tiles: chunk granularity so attention/proj can
            # start as soon as their inputs exist.
            KT = [[pers.tile([128, 512], F16, tag=f"kt{i}_{c}", name=f"kt{i}_{c}")
                   for c in range(4)] for i in range(8)]
            QT = [[pers.tile([128, 512], F16, tag=f"qt{i}_{c}", name=f"qt{i}_{c}")
                   for c in range(2)] for i in range(8)]
            VA = [pers.tile([128, 4, H, 65], F16, tag=f"va{c}", name=f"va{c}")
                  for c in range(4)]
            CN = [[pers.tile([128, 256], F16, tag=f"cn{i}_{c}", name=f"cn{i}_{c}")
                   for c in range(NSPAN)] for i in range(8)]

            for c in range(4):
                nc.gpsimd.memset(VA[c][:, :, :, 64:65], 1.0)

            mk = pers.tile([128, NSPAN, 768], F32)
            nc.sync.dma_start(mk[:], masks[:])
            scv = pers.tile([128, NSPAN, KB], F32)
            nc.sync.dma_start(scv[:], scv_d[:])
            biv = pers.tile([128, NSPAN, KB], F32)
            nc.sync.dma_start(biv[:], biv_d[:])
            wpt = pers.tile([128, 8, E], F16)
            nc.gpsimd.dma_start(wpt[:], wp_r[:])
            bpb = pers.tile([128, E], F32)
            nc.sync.dma_start(bpb[:], bpv[0:1, :].to_broadcast((128, E)))

            def emit_qt(qh):
                xqs = ph1.tile([128, 8, 512], F16, tag="xts")
                nc.gpsimd.dma_start(xqs[:], xqt_r[:, :, qh * 512:(qh + 1) * 512])
                for hp in range(8):
                    wq_t = ph1.tile([128, 8, 128], F16, tag="wk")
                    nc.gpsimd.dma_start(wq_t[:], wqt_r[:, :, hp * 128:(hp + 1) * 128])
                    qps = psA.tile([128, 512], F32, tag="pps")
                    for ec in range(8):
                        nc.tensor.matmul(qps[:], wq_t[:, ec, :], xqs[:, ec, :],
                                         start=(ec == 0), stop=(ec == 7))
                    nc.vector.tensor_copy(QT[hp][qh][:], qps[:])

            def emit_kv(sp4):
                xts = ph1.tile([128, 8, 512], F16, tag="xts")
                nc.gpsimd.dma_start(xts[:], xt_r[:, :, sp4 * 512:(sp4 + 1) * 512])
                for hp in range(8):
                    wk_t = ph1.tile([128, 8, 128], F16, tag="wk")
                    nc.gpsimd.dma_start(wk_t[:], wkt_r[:, :, hp * 128:(hp + 1) * 128])
                    kps = psA.tile([128, 512], F32, tag="pps")
                    for ec in range(8):
                        nc.tensor.matmul(kps[:], wk_t[:, ec, :], xts[:, ec, :],
                                         start=(ec == 0), stop=(ec == 7))
                    nc.vector.tensor_copy(KT[hp][sp4][:], kps[:])
                for hh in range(2):
                    wv_t = ph1.tile([128, 8, 512], F16, tag="wv")
                    nc.gpsimd.dma_start(wv_t[:], wvt_r[:, :, hh * 512:(hh + 1) * 512])
                    for j in range(4):
                        vps = psA.tile([128, 512], F32, tag="pps")
                        for ec in range(8):
                            nc.tensor.matmul(vps[:], xts[:, ec, j * 128:(j + 1) * 128],
                                             wv_t[:, ec, :],
                                             start=(ec == 0), stop=(ec == 7))
                        for hl in range(8):
                            h = hh * 8 + hl
                            nc.vector.tensor_copy(VA[sp4][:, j, h, 0:64],
                                                  vps[:, hl * 64:hl * 64 + 64])

            def emit_span(s):
                Bs = BOUNDS[s]
                q0 = s * 256
                qh, qo = s // 2, (s % 2) * 256
                for gg in range(8):
                    # group of 2 same-parity heads: j = gg // 2, p = gg % 2
                    # heads (4j+p, 4j+2+p) -> KT/QT pair rows p*64..p*64+64
                    j, p = gg // 2, gg % 2
                    hA, hB = 4 * j + p, 4 * j + 2 + p
                    r0 = p * 64
                    tp = (64, 0) if p else None
                    cpsA = psC.tile([65, 256], F32, tag="cpsA")
                    cpsB = psC.tile([65, 256], F32, tag="cpsB")
                    for kb in range(Bs):
                        sps = psS.tile([128, 2, 256], F32, tag="sps")
                        c4, k0 = kb // 4, (kb % 4) * 128
                        nc.tensor.matmul(sps[:, 0, :],
                                         KT[2 * j][c4][r0:r0 + 64, k0:k0 + 128],
                                         QT[2 * j][qh][r0:r0 + 64, qo:qo + 256],
                                         start=True, stop=True, tile_position=tp)
                        nc.tensor.matmul(sps[:, 1, :],
                                         KT[2 * j + 1][c4][r0:r0 + 64, k0:k0 + 128],
                                         QT[2 * j + 1][qh][r0:r0 + 64, qo:qo + 256],
                                         start=True, stop=True, tile_position=tp)
                        m = kb - (Bs - 4)
                        if m >= 0:
                            w = MASK_W[m]
                            off = MASK_OFFS[m]
                            mkap = mk[:, s, off:off + w] \
                                .rearrange("p (o w) -> p o w", o=1) \
                                .to_broadcast((128, 2, w))
                            nc.vector.tensor_add(sps[:, 0:2, 0:w], sps[:, 0:2, 0:w], mkap)
                        pt = ph2.tile([128, 2, 256], F16, tag="pt")
                        nc.scalar.activation(pt[:], sps[:], mybir.ActivationFunctionType.Exp,
                                             scale=scv[:, s, kb:kb + 1],
                                             bias=biv[:, s, kb:kb + 1])
                        st, en = (kb == 0), (kb == Bs - 1)
                        nc.tensor.matmul(cpsA[:], VA[c4][:, kb % 4, hA, :], pt[:, 0, :],
                                         start=st, stop=en)
                        nc.tensor.matmul(cpsB[:], VA[c4][:, kb % 4, hB, :], pt[:, 1, :],
                                         start=st, stop=en)
                    # stage out of PSUM quickly, then normalize from SBUF
                    ctxu = ph2c.tile([65, 2, 256], F32, tag="ctxu")
                    nc.vector.tensor_copy(ctxu[:, 0, :], cpsA[:])
                    nc.vector.tensor_copy(ctxu[:, 1, :], cpsB[:])
                    rs = ph2c.tile([1, 2, 256], F32, tag="rs")
                    nc.vector.tensor_copy(rs[0:1, 0, :], ctxu[64:65, 0, :])
                    nc.vector.tensor_copy(rs[0:1, 1, :], ctxu[64:65, 1, :])
                    rr = ph2c.tile([1, 2, 256], F32, tag="rr")
                    nc.vector.reciprocal_approx_fast(rr[:], rs[:])
                    rd = dram.tile([1, 2, 256], F32, tag="rd")
                    nc.sync.dma_start(rd[:], rr[:])
                    for i, h in ((0, hA), (1, hB)):
                        bct = ph2c.tile([64, 256], F32, tag="bct")
                        nc.sync.dma_start(bct[:], rd[0:1, i, :].to_broadcast((64, 256)))
                        hp2, rr0 = h // 2, (h % 2) * 64
                        nc.vector.tensor_mul(CN[hp2][s][rr0:rr0 + 64, :],
                                             ctxu[0:64, i, :], bct[:])

            def emit_proj(s):
                for qb in (2 * s, 2 * s + 1):
                    for eo in range(2):
                        ops = psO.tile([128, 512], F32, tag="ops")
                        for c in range(8):
                            nc.tensor.matmul(ops[:],
                                             CN[c][s][:, (qb % 2) * 128:(qb % 2) * 128 + 128],
                                             wpt[:, c, eo * 512:(eo + 1) * 512],
                                             start=(c == 0), stop=(c == 7))
                        ot = ph3.tile([128, 512], F32, tag="ot")
                        nc.vector.tensor_add(ot[:], ops[:], bpb[:, eo * 512:(eo + 1) * 512])
                        nc.sync.dma_start(out[qb * 128:(qb + 1) * 128,
                                              eo * 512:(eo + 1) * 512], ot[:])

            emit_qt(0)
            emit_kv(0)
            emit_span(0)
            emit_proj(0)
            emit_kv(1)
            emit_span(1)
            emit_proj(1)
            emit_qt(1)
            emit_kv(2)
            emit_span(2)
            emit_proj(2)
            emit_kv(3)
            emit_span(3)
            emit_proj(3)
    nc.compile()
    return nc


_NC_CACHE = None


def _host_side_tables(half):
    """Triangular masks, scale and bias vectors for one core half."""
    mask = np.zeros((128, NSPAN, 768), np.float32)
    scv = np.zeros((128, NSPAN, KB), np.float32)
    biv = np.zeros((128, NSPAN, KB), np.float32)
    for s in range(NSPAN):
        Bs = BOUNDS[s]
        gmax = OWN_BLOCKS[half][s][1]
        for kb in range(Bs):
            if kb > gmax:
                scv[:, s, kb] = 0.0
                biv[:, s, kb] = -30.0
            else:
                scv[:, s, kb] = 1.0 / np.sqrt(D)
                biv[:, s, kb] = 0.0
        for m in range(4):
            kb = Bs - 4 + m
            w = MASK_W[m]
            off = MASK_OFFS[m]
            nqb = m % 2 + 1   # q-blocks covered by this mask
            for jj in range(nqb):
                g = OWN_BLOCKS[half][s][jj]
                kpos = kb * 128 + np.arange(128)[:, None]
                qpos = g * 128 + np.arange(128)[None, :]
                mask[:, s, off + jj * 128: off + (jj + 1) * 128] = \
                    np.where(qpos >= kpos, 0.0, -240.0)
    return mask, scv, biv


def kernel(x, Wq, Wk, Wv, Wp, bp):
    global _NC_CACHE
    x = np.asarray(x, np.float32)
    Wq = np.asarray(Wq, np.float32)
    Wk = np.asarray(Wk, np.float32)
    Wv = np.asarray(Wv, np.float32)
    Wp = np.asarray(Wp, np.float32)
    bp = np.asarray(bp, np.float32)

    if _NC_CACHE is None:
        _NC_CACHE = _build_nc()
    nc = _NC_CACHE

    wqt = np.ascontiguousarray(Wq.transpose(1, 0, 2).reshape(E, E))
    wkt = np.ascontiguousarray(Wk.transpose(1, 0, 2).reshape(E, E))
    wvt = np.ascontiguousarray(Wv.transpose(1, 0, 2).reshape(E, E))
    wp_c = np.ascontiguousarray(Wp)
    bpv = bp.reshape(1, E)
    tables = {h: _host_side_tables(h) for h in (0, 1)}
    rows = {h: own_rows(h) for h in (0, 1)}

    in_maps = []
    for c in range(8):
        b, h = c // 2, c % 2
        xb = x[b]
        mask, scv, biv = tables[h]
        in_maps.append({
            "xt": np.ascontiguousarray(xb.T),
            "xqt": np.ascontiguousarray(xb[rows[h]].T),
            "wqt": wqt, "wkt": wkt, "wvt": wvt, "wp": wp_c, "bpv": bpv,
            "masks": mask, "scv": scv, "biv": biv,
        })

    trace = bool(os.environ.get("BASS_ATTN_TRACE"))
    res = run_bass_kernel_spmd(nc, in_maps, core_ids=list(range(8)), trace=trace)
    if trace and res.exec_time_ns is not None:
        print(f"HW exec time: {res.exec_time_ns} ns")
        for scope, cores in sorted((res.per_core_scope_times or {}).items()):
            print("scope", scope, cores)
        if res.instructions_and_trace:
            print("trace path:", res.instructions_and_trace[1])

    out = np.empty((B, S, E), np.float32)
    for c in range(8):
        b, h = c // 2, c % 2
        out[b, rows[h]] = res.results[c]["out"]
    return out
